# revision 7
# baseline (speedup 1.0000x reference)
"""LocalityEnhancedAttention Trainium2 kernel (8 NeuronCores, SPMD).

Sharding: core c handles batch b = c//2 and head-group g = c%2
(8 of 16 heads). Each core computes its partial output projection
(attn_heads @ wo_shard.T); host sums the two head-group partials per
batch and adds bo.

Device dataflow per core (S=2048, M=1024, local head-dims DH=512):
  - host pre-transposes inputs:  qT/kT/vT = x[b].T  [M, S]
  - projections (f32r matmuls): Q^T,K^T [DH, S] in [d, s] layout,
    V in [s, d] layout augmented with a ones column per head (rowsums)
  - scores^T[kj, qi] = K^T.T @ Q^T per head, head pairs packed into
    PE row-halves (contraction = Dk = 64), banded local bias added via
    DVE, exp via ACT (scale=1/8 folded in), P^T in SBUF
  - PV: A^T_aug[65, qi] += V_aug[kj].T @ P^T[kj] accumulated in PSUM;
    row 64 = softmax denominators.  Normalize via DVE recip +
    gpsimd partition_broadcast + DVE mul.
  - WO: out_partial[s, m] = sum_d A_norm^T.T @ woT
"""

import os
import sys
from contextlib import ExitStack

import numpy as np

sys.path.insert(0, "/opt/trn_rl_repo")

import ml_dtypes

BF = ml_dtypes.bfloat16

import concourse.bass as bass
import concourse.mybir as mybir
import concourse.tile as tile
from concourse import bacc
from concourse.bass_utils import run_bass_kernel_spmd

F32 = mybir.dt.float32
F32R = mybir.dt.float32r
BF16 = mybir.dt.bfloat16
EXP = mybir.ActivationFunctionType.Exp
TS, DS = bass.ts, bass.ds

S = 2048
M = 1024
DH = 512        # head dims per core (8 heads x 64)
DK = 64
W = 16
NPT = 4         # head pairs per core
NCH = 4         # qi chunks of 512
NKJ = 16        # kj tiles of 128


def _emit(ctx, tc, io):
    nc = tc.nc

    const = ctx.enter_context(tc.tile_pool(name="const", bufs=1))
    qkvp = ctx.enter_context(tc.tile_pool(name="qkv", bufs=1))
    ap_ = ctx.enter_context(tc.tile_pool(name="anorm", bufs=1))
    wop = ctx.enter_context(tc.tile_pool(name="wop", bufs=1))

    pat = const.tile([128, 160], F32, tag="pat", name="pat")
    nc.sync.dma_start(pat[:], io["pat"])
    ones_r = const.tile([1, 512], BF16, tag="ones_r", name="ones_r")
    nc.vector.memset(ones_r[:], 1.0)
    bq = const.tile([1, DH], BF16, tag="bq", name="bq")
    nc.sync.dma_start(bq[:], io["bq"])
    bk = const.tile([1, DH], BF16, tag="bk", name="bk")
    nc.sync.dma_start(bk[:], io["bk"])
    bv = const.tile([1, DH], BF16, tag="bv", name="bv")
    nc.sync.dma_start(bv[:], io["bv"])

    qT_sb = [qkvp.tile([128, S], BF16, tag=f"q{i}", name=f"q{i}") for i in range(NPT)]
    kT_sb = [qkvp.tile([128, S], BF16, tag=f"k{i}", name=f"k{i}") for i in range(NPT)]
    v_sb = [qkvp.tile([128, 8 * 65], BF16, tag=f"v{i}", name=f"v{i}") for i in range(16)]
    a_sb = [ap_.tile([128, S], BF16, tag=f"a{i}", name=f"a{i}") for i in range(NPT)]
    woT_sb = [wop.tile([128, M], BF16, tag=f"wo{i}", name=f"wo{i}") for i in range(NPT)]
    for i in range(NPT):
        nc.sync.dma_start(woT_sb[i][:], io["woT"][TS(i, 128), :])

    # ---------------- projections ----------------
    with ExitStack() as ps:
        wpool = ps.enter_context(tc.tile_pool(name="wpool", bufs=10))
        stream = ps.enter_context(tc.tile_pool(name="stream", bufs=12))
        pproj = ps.enter_context(tc.tile_pool(name="pproj", bufs=2, space="PSUM"))

        # V projection -> v_sb[st] = [128, 8*65] ([s, head-major d | ones])
        wv = []
        for k in range(8):
            t = wpool.tile([128, DH], BF16, tag="w", name="w")
            nc.sync.dma_start(t[:], io["wvT"][TS(k, 128), :])
            wv.append(t)
        for st in range(16):
            vv = v_sb[st].rearrange("p (h e) -> p h e", e=65)
            nc.vector.memset(vv[:, :, 64:65], 1.0)
        for sc in range(4):  # s-chunks of 512
            vs = []
            for k in range(8):
                t = stream.tile([128, 512], BF16, tag="s", name="s")
                nc.sync.dma_start(t[:], io["vT"][TS(k, 128), TS(sc, 512)])
                vs.append(t)
            for j in range(4):
                st = sc * 4 + j
                psv = pproj.tile([128, DH], F32, tag="pp", name="pp")
                for k in range(8):
                    nc.tensor.matmul(
                        psv[:],
                        lhsT=vs[k][:, TS(j, 128)],
                        rhs=wv[k][:],
                        start=(k == 0), stop=False, skip_group_check=True)
                nc.tensor.matmul(
                    psv[:],
                    lhsT=ones_r[0:1, 0:128],
                    rhs=bv[:],
                    start=False, stop=True, skip_group_check=True)
                vv = v_sb[st].rearrange("p (h e) -> p h e", e=65)
                pv_view = psv.rearrange("p (h e) -> p h e", e=64)
                nc.vector.tensor_copy(vv[:, :, 0:64], pv_view[:, :, :])

        # Q^T and K^T projections -> [d, s] layout pair tiles
        for (w_name, x_name, b_tile, dst) in (
            ("wqT", "qT", bq, qT_sb),
            ("wkT", "kT", bk, kT_sb),
        ):
            wt = []
            for k in range(8):
                t = wpool.tile([128, DH], BF16, tag="w", name="w")
                nc.sync.dma_start(t[:], io[w_name][TS(k, 128), :])
                wt.append(t)
            for sc in range(4):
                xs = []
                for k in range(8):
                    t = stream.tile([128, 512], BF16, tag="s", name="s")
                    nc.sync.dma_start(t[:], io[x_name][TS(k, 128), TS(sc, 512)])
                    xs.append(t)
                for pt in range(NPT):
                    psq = pproj.tile([128, 512], F32, tag="pp", name="pp")
                    for k in range(8):
                        nc.tensor.matmul(
                            psq[:],
                            lhsT=wt[k][:, TS(pt, 128)],
                            rhs=xs[k][:],
                            start=(k == 0), stop=False, skip_group_check=True)
                    nc.tensor.matmul(
                        psq[:],
                        lhsT=b_tile[0:1, TS(pt, 128)],
                        rhs=ones_r[:],
                        start=False, stop=True, skip_group_check=True)
                    nc.vector.tensor_copy(dst[pt][:, TS(sc, 512)], psq[:])

    # ---------------- attention ----------------
    with ExitStack() as asx:
        ptp = asx.enter_context(tc.tile_pool(name="ptp", bufs=4))
        rp = asx.enter_context(tc.tile_pool(name="rp", bufs=6))
        sps = asx.enter_context(tc.tile_pool(name="sps", bufs=3, space="PSUM"))
        pvs = asx.enter_context(tc.tile_pool(name="pvs", bufs=2, space="PSUM"))

        for pt in range(NPT):
            for ch in range(NCH):
                pvt = [pvs.tile([65, 512], F32, tag="pv", name="pv") for _ in range(2)]
                c0 = ch * 512
                for kj in range(16):
                    kj0 = kj * 128
                    sp = sps.tile([128, 1024], F32, tag="sp", name="sp")
                    for h in (0, 1):
                        nc.tensor.matmul(
                            sp[:, DS(h * 512, 512)],
                            lhsT=kT_sb[pt][DS(h * 64, 64), TS(kj, 128)],
                            rhs=qT_sb[pt][DS(h * 64, 64), TS(ch, 512)],
                            start=True, stop=True,
                            tile_position=(h * 64, 0),
                            skip_group_check=True)
                    lo = max(kj0 - W, c0)
                    hi = min(kj0 + 128 + W, c0 + 512)
                    if lo < hi:
                        pa = lo - (kj0 - W)
                        for h in (0, 1):
                            sl = sp[:, DS(h * 512 + lo - c0, hi - lo)]
                            nc.vector.tensor_add(sl, sl, pat[:, DS(pa, hi - lo)])
                    ptt = ptp.tile([128, 1024], BF16, tag="ptt", name="ptt")
                    nc.scalar.activation(ptt[:], sp[:], EXP, scale=0.125)
                    for h in (0, 1):
                        lh = pt * 2 + h
                        nc.tensor.matmul(
                            pvt[h][:],
                            lhsT=v_sb[kj][:, DS(lh * 65, 65)],
                            rhs=ptt[:, DS(h * 512, 512)],
                            start=(kj == 0), stop=(kj == 15),
                            skip_group_check=True)
                for h in (0, 1):
                    r1 = rp.tile([1, 512], F32, tag="r1", name="r1")
                    nc.vector.reciprocal(r1[:], pvt[h][DS(64, 1), :])
                    rb = rp.tile([64, 512], F32, tag="rb", name="rb")
                    nc.gpsimd.partition_broadcast(rb[:], r1[:])
                    nc.vector.tensor_mul(
                        a_sb[pt][DS(h * 64, 64), TS(ch, 512)],
                        pvt[h][DS(0, 64), :], rb[:])

    # ---------------- output projection ----------------
    with ExitStack() as wos:
        wops = wos.enter_context(tc.tile_pool(name="wops", bufs=2, space="PSUM"))
        op = wos.enter_context(tc.tile_pool(name="op", bufs=4))
        for st in range(16):
            for mt in range(2):
                pso = wops.tile([128, 512], F32, tag="pso", name="pso")
                for pt in range(NPT):
                    nc.tensor.matmul(
                        pso[:],
                        lhsT=a_sb[pt][:, TS(st, 128)],
                        rhs=woT_sb[pt][:, TS(mt, 512)],
                        start=(pt == 0), stop=(pt == 3),
                        skip_group_check=True)
                ot = op.tile([128, 512], F32, tag="ot", name="ot")
                nc.vector.tensor_copy(ot[:], pso[:])
                nc.sync.dma_start(io["out"][TS(st, 128), TS(mt, 512)], ot[:])


_CACHE = {}


def _build():
    if "nc" in _CACHE:
        return _CACHE["nc"]
    nc = bacc.Bacc("TRN2", target_bir_lowering=False, debug=False)
    io = {}
    for name, shape in (
        ("qT", [M, S]), ("kT", [M, S]), ("vT", [M, S]),
        ("wqT", [M, DH]), ("wkT", [M, DH]), ("wvT", [M, DH]),
        ("woT", [DH, M]),
        ("bq", [1, DH]), ("bk", [1, DH]), ("bv", [1, DH]),    ):
        io[name] = nc.dram_tensor(name, shape, BF16, kind="ExternalInput").ap()
    io["pat"] = nc.dram_tensor("pat", [128, 160], F32, kind="ExternalInput").ap()
    io["out"] = nc.dram_tensor("out", [S, M], F32, kind="ExternalOutput").ap()
    with tile.TileContext(nc) as tc:
        with ExitStack() as ctx:
            _emit(ctx, tc, io)
    nc.compile()
    _CACHE["nc"] = nc
    return nc


def _bias_pattern(local_bias):
    # pattern[p, f] covers scores^T tile rows kj0+p, cols qi = kj0-16+f.
    # rel = qi - kj = f - 16 - p; bias term = LOCALITY_STRENGTH(2) * b[rel+16],
    # pre-scaled by 16 because ACT folds in scale=1/8: exp((s + 16*2*b)/8).
    p = np.arange(128)[:, None]
    f = np.arange(160)[None, :]
    idx = f - p  # rel + 16
    valid = (idx >= 0) & (idx <= 2 * W)
    pat = np.where(valid, 16.0 * np.asarray(local_bias, np.float32)[np.clip(idx, 0, 2 * W)], 0.0)
    return np.ascontiguousarray(pat, dtype=np.float32)


def kernel(query, key, value, wq, bq, wk, bk, wv, bv, wo, bo, local_bias):
    query = np.asarray(query, np.float32)
    key = np.asarray(key, np.float32)
    value = np.asarray(value, np.float32)
    wq, wk, wv, wo = (np.asarray(x, np.float32) for x in (wq, wk, wv, wo))
    bq, bk, bv, bo = (np.asarray(x, np.float32) for x in (bq, bk, bv, bo))
    pat = _bias_pattern(local_bias)

    nc = _build()
    in_maps = []
    for c in range(8):
        b, g = c // 2, c % 2
        sl = slice(g * DH, (g + 1) * DH)
        in_maps.append({
            "qT": np.ascontiguousarray(query[b].T).astype(BF),
            "kT": np.ascontiguousarray(key[b].T).astype(BF),
            "vT": np.ascontiguousarray(value[b].T).astype(BF),
            "wqT": np.ascontiguousarray(wq[sl, :].T).astype(BF),
            "wkT": np.ascontiguousarray(wk[sl, :].T).astype(BF),
            "wvT": np.ascontiguousarray(wv[sl, :].T).astype(BF),
            "woT": np.ascontiguousarray(wo[:, sl].T).astype(BF),
            "bq": np.ascontiguousarray(bq[sl]).reshape(1, DH).astype(BF),
            "bk": np.ascontiguousarray(bk[sl]).reshape(1, DH).astype(BF),
            "bv": np.ascontiguousarray(bv[sl]).reshape(1, DH).astype(BF),
            "pat": pat,
        })
    res = run_bass_kernel_spmd(
        nc, in_maps, core_ids=list(range(8)),
        trace=bool(int(os.environ.get("KERNEL_TRACE", "0"))),
    )
    _CACHE["last_result"] = res
    outs = [r["out"] for r in res.results]
    out = np.stack([outs[2 * b] + outs[2 * b + 1] + bo for b in range(4)])
    return out.astype(np.float32)
